# revision 62
# baseline (speedup 1.0000x reference)
"""Multi-head attention block (B=8, N=1024, D=768, H=12 heads) on 8 trn2 NeuronCores.

Sharding: pure data-parallel over the batch dimension (one batch element per
core, weights replicated). No collectives needed.

Per-core kernel (Bass/Tile, all matmuls in float32r ~= TF32 precision):
  xT = x.T                      (PE transpose, [768, 1024])
  qT,kT = (W_q|W_k).T-proj      ([feature, token] layout, 12 x 128-row tiles)
  v = x @ W_v                   (natural [token, feature] layout + ones column)
  per head: S^T = kT.T-contract-qT tiles, P^T = exp(0.125*S^T)  (no max-sub:
            logits ~ N(0,1), exp range is tiny)
  out^T[d+1, n_q] = [v|1].T @ P^T  accumulated over k-tiles; row 64 = softmax
            denominator; normalize via reciprocal + DMA partition-broadcast
  y = out.T @ W_proj + b_proj   (natural layout via outT-stationary matmuls)
"""

import numpy as np

B, N, D = 8, 1024, 768
NH, HD = 12, 64
SCALE = HD ** -0.5  # 0.125
NT = N // 128       # 8 token tiles
NKT = D // 128      # 6 contraction tiles over D
NHP = NH // 2       # 6 head pairs

_STATE = {}


def _build():
    import concourse.bacc as bacc
    import concourse.bass as bass
    import concourse.mybir as mybir
    from concourse import tile
    from concourse.masks import make_identity

    f32 = mybir.dt.float32
    f32r = mybir.dt.float32r
    EXP = mybir.ActivationFunctionType.Exp

    nc = bacc.Bacc(None, target_bir_lowering=False)
    x = nc.dram_tensor("x", [N, D], f32, kind="ExternalInput")
    wqkv = nc.dram_tensor("w_qkv", [D, 3 * D], f32r, kind="ExternalInput")
    wproj = nc.dram_tensor("w_proj", [D, D], f32r, kind="ExternalInput")
    bproj = nc.dram_tensor("b_proj", [D], f32, kind="ExternalInput")
    y = nc.dram_tensor("y", [N, D], f32, kind="ExternalOutput")
    den_dram = nc.dram_tensor("den_scratch", [NH, N], f32)

    with tile.TileContext(nc) as tc:
        with (
            tc.tile_pool(name="const", bufs=1) as const,
            tc.tile_pool(name="big", bufs=1) as big,
            tc.tile_pool(name="wp_pool", bufs=1) as wp_pool,
            tc.tile_pool(name="ystage", bufs=3) as ystage,
        ):
            ident = const.tile([128, 128], f32)
            make_identity(nc, ident[:])
            zb = const.tile([128, 1], f32)
            nc.vector.memset(zb[:], 0.0)
            onef = const.tile([128, 1], f32)
            nc.vector.memset(onef[:], 1.0)
            # b_proj broadcast across partitions (zero-stride DRAM source)
            bias_bc = const.tile([128, D], f32)
            nc.gpsimd.dma_start(
                out=bias_bc[:],
                in_=bass.AP(tensor=bproj, offset=0, ap=[[0, 128], [1, D]]),
            )

            # persistent activations
            qkT = big.tile([128, 2 * NHP, N], f32r)       # q ftiles 0..5, k ftiles 6..11
            vban = big.tile([128, NT, NH, HD + 1], f32r)  # v natural + ones col
            outT = big.tile([128, NHP, N], f32r)          # attention out, transposed

            wp_sb = wp_pool.tile([128, NKT, D], f32r)

            # ones columns for the denominator trick (value cols written by the
            # v-eviction copies below)
            nc.vector.tensor_copy(
                vban[:, :, :, HD:HD + 1].rearrange("p a b one -> p (a b one)"),
                onef[:, 0:1].to_broadcast((128, NT * NH)),
            )

            # ---- Phase 1: transpose x interleaved with v projection (v of
            #      token tile tt only needs tt's transposes, so emit v one
            #      tile behind the transposes) ----
            with tc.tile_pool(name="xt_pool", bufs=1) as xt_pool:
                xT = xt_pool.tile([128, NKT, N], f32r)
                with (
                    tc.tile_pool(name="stage", bufs=3) as stage,
                    tc.tile_pool(name="wv_pool", bufs=1) as wv_pool,
                    tc.tile_pool(name="ps_t", bufs=3, space="PSUM") as ps_t,
                    tc.tile_pool(name="ps_v", bufs=2, space="PSUM") as ps_v,
                ):
                    wv_sb = wv_pool.tile([128, NKT, D], f32r)

                    def emit_v(tt):
                        psv0 = ps_v.tile([128, 384], f32, tag="vps0", name="psv0")
                        psv1 = ps_v.tile([128, 384], f32, tag="vps1", name="psv1")
                        psvs = (psv0, psv1)
                        for kt in range(NKT):
                            for fc in range(2):
                                nc.tensor.matmul(
                                    psvs[fc][:],
                                    xT[:, kt, tt * 128:(tt + 1) * 128],
                                    wv_sb[:, kt, fc * 384:(fc + 1) * 384],
                                    start=(kt == 0),
                                    stop=(kt == NKT - 1),
                                )
                        for fc in range(2):
                            nc.vector.tensor_copy(
                                vban[:, tt, fc * 6:(fc + 1) * 6, 0:HD],
                                psvs[fc][:].rearrange("p (h d) -> p h d", h=6),
                            )

                    for tt in range(NT):
                        xst = stage.tile([128, D], f32, tag="xst", name="xst")
                        nc.sync.dma_start(xst[:], x[tt * 128:(tt + 1) * 128, :])
                        if tt in (0, 1):
                            for kt in range(3 * tt, 3 * tt + 3):
                                nc.sync.dma_start(
                                    wv_sb[:, kt, :],
                                    wqkv[kt * 128:(kt + 1) * 128, 2 * D:3 * D],
                                )
                        for dt_ in range(NKT):
                            pst = ps_t.tile([128, 128], f32, tag="tps", name="pst")
                            nc.tensor.transpose(
                                pst[:], xst[:, dt_ * 128:(dt_ + 1) * 128], ident[:]
                            )
                            # ScalarE is idle in the prologue - use it to evict
                            nc.scalar.copy(
                                xT[:, dt_, tt * 128:(tt + 1) * 128], pst[:]
                            )
                        if tt >= 1:
                            emit_v(tt - 1)
                    emit_v(NT - 1)

                # ---- attention, with q/k projection steps interleaved ----
                with (
                    tc.tile_pool(name="wq_pool", bufs=16) as wq_pool,
                    tc.tile_pool(name="pt_pool", bufs=4) as pt_pool,
                    tc.tile_pool(name="s_ps", bufs=2, space="PSUM") as s_ps,
                    tc.tile_pool(name="acc_ps", bufs=1, space="PSUM") as acc_ps,
                    tc.tile_pool(name="qk_ps", bufs=1, space="PSUM") as qk_ps,
                    tc.tile_pool(name="norm", bufs=2) as norm,
                ):
                    def qk_steps_for(ft, evict_on_scalar=False, use_s_slot=False):
                        """Generator of closures; each emits one PE step of the
                        qT/kT projection for feature tile ft (2 psum halves)."""
                        if use_s_slot:
                            # prologue only: the s_ps slots are idle, and one
                            # [128,1024] slot per feature tile lets consecutive
                            # tiles pipeline instead of serializing on psq0/1
                            psq01 = s_ps.tile([128, 1024], f32, tag="s",
                                              name="psq01")
                            psqs = (psq01[:, 0:512], psq01[:, 512:1024])
                        else:
                            psq0 = qk_ps.tile([128, 512], f32, tag="psq0",
                                              name="psq0")
                            psq1 = qk_ps.tile([128, 512], f32, tag="psq1",
                                              name="psq1")
                            psqs = (psq0[:], psq1[:])

                        def mk_mm(kt):
                            def emit():
                                wt = wq_pool.tile(
                                    [128, 128], f32r, tag="wt", name="wt"
                                )
                                nc.sync.dma_start(
                                    wt[:],
                                    wqkv[kt * 128:(kt + 1) * 128,
                                         ft * 128:(ft + 1) * 128],
                                )
                                for qch in range(2):
                                    nc.tensor.matmul(
                                        psqs[qch][:],
                                        wt[:],
                                        xT[:, kt, qch * 512:(qch + 1) * 512],
                                        start=(kt == 0),
                                        stop=(kt == NKT - 1),
                                    )
                            return emit

                        def mk_evict():
                            def emit():
                                for qch in range(2):
                                    if evict_on_scalar:
                                        nc.scalar.copy(
                                            qkT[:, ft, qch * 512:(qch + 1) * 512],
                                            psqs[qch][:],
                                        )
                                    else:
                                        nc.vector.tensor_copy(
                                            qkT[:, ft, qch * 512:(qch + 1) * 512],
                                            psqs[qch][:],
                                        )
                            return emit

                        return [mk_mm(kt) for kt in range(NKT)] + [mk_evict()]

                    def proj_steps_for(tt):
                        """Output projection of token tile tt as interleavable
                        steps (borrows the idle qk PSUM slots)."""
                        psy0 = qk_ps.tile([128, 512], f32, tag="psq0",
                                          name="psy0")
                        psy1 = qk_ps.tile([128, 512], f32, tag="psq1",
                                          name="psy1")
                        psys = (psy0, psy1)

                        def mk_mm(j0):
                            def emit():
                                for j in (j0, j0 + 1):
                                    for fc in range(2):
                                        nc.tensor.matmul(
                                            psys[fc][:, 0:384],
                                            outT[:, j, tt * 128:(tt + 1) * 128],
                                            wp_sb[:, j, fc * 384:(fc + 1) * 384],
                                            start=(j == 0),
                                            stop=(j == NHP - 1),
                                        )
                            return emit

                        def mk_evict():
                            def emit():
                                for fc in range(2):
                                    yst = ystage.tile([128, 384], f32, tag="yst",
                                                      name="yst")
                                    nc.vector.tensor_add(
                                        yst[:], psys[fc][:, 0:384],
                                        bias_bc[:, fc * 384:(fc + 1) * 384],
                                    )
                                    nc.sync.dma_start(
                                        y[tt * 128:(tt + 1) * 128,
                                          fc * 384:(fc + 1) * 384],
                                        yst[:],
                                    )
                            return emit

                        return [mk_mm(j0) for j0 in range(0, NHP, 2)] + [mk_evict()]

                    # prologue: q/k feature tiles for head pair 0 (ScalarE is
                    # idle here - use it for the PSUM evictions)
                    for step in (qk_steps_for(0, evict_on_scalar=True,
                                              use_s_slot=True)
                                 + qk_steps_for(NHP, evict_on_scalar=True,
                                                use_s_slot=True)):
                        step()

                    for hp in range(NHP):
                        # stream one W_proj k-tile per head pair (ready by proj)
                        nc.sync.dma_start(
                            wp_sb[:, hp, :], wproj[hp * 128:(hp + 1) * 128, :]
                        )
                        # qk steps for the next head pair, spread across this
                        # head pair's 16 attention chunks
                        pending = []
                        if hp + 1 < NHP:
                            pending = qk_steps_for(hp + 1) + qk_steps_for(
                                NHP + hp + 1
                            )
                        for qc in range(2):
                            if hp == NHP - 1 and qc == 1:
                                # outT for tokens 0..511 is complete after
                                # (hp5, qc0): pre-compute their projection here
                                for tt_ in range(4):
                                    pending += proj_steps_for(tt_)
                            acc0 = acc_ps.tile([HD + 1, 512], f32, tag="acc0",
                                               name="acc0")
                            acc1 = acc_ps.tile([HD + 1, 512], f32, tag="acc1",
                                               name="acc1")
                            accs = (acc0, acc1)
                            pts = []

                            def emit_av(kt):
                                pt = pts[kt]
                                for h in range(2):
                                    nc.tensor.matmul(
                                        accs[h][:],
                                        vban[:, kt, hp * 2 + h, :],
                                        pt[:, h * 512:(h + 1) * 512],
                                        start=(kt == 0),
                                        stop=(kt == NT - 1),
                                    )

                            for kt in range(NT):
                                ssum = s_ps.tile([128, 1024], f32, tag="s",
                                                 name="ssum")
                                for h in range(2):
                                    ksl = qkT[h * 64:(h + 1) * 64, NHP + hp,
                                              kt * 128:(kt + 1) * 128]
                                    qsl = qkT[h * 64:(h + 1) * 64, hp,
                                              qc * 512:(qc + 1) * 512]
                                    nc.tensor.matmul(
                                        ssum[:, h * 512:(h + 1) * 512],
                                        ksl,
                                        qsl,
                                        start=True,
                                        stop=True,
                                    )
                                pt = pt_pool.tile([128, 1024], f32r, tag="pt",
                                                  name="pt")
                                nc.scalar.activation(
                                    pt[:], ssum[:], EXP, bias=zb[:], scale=SCALE
                                )
                                pts.append(pt)
                                if kt >= 1:
                                    emit_av(kt - 1)
                                for _ in range(2 if hp == NHP - 1 else 1):
                                    if pending:
                                        pending.pop(0)()
                            emit_av(NT - 1)

                            # evict accumulators to SBUF immediately (single
                            # PSUM reader -> acc banks free early), then
                            # normalize entirely from SBUF
                            asb0 = norm.tile([HD + 1, 512], f32, tag="asb0",
                                             name="asb0")
                            asb1 = norm.tile([HD + 1, 512], f32, tag="asb1",
                                             name="asb1")
                            asbs = (asb0, asb1)
                            nc.vector.tensor_copy(asb0[:], acc0[:])
                            nc.vector.tensor_copy(asb1[:], acc1[:])
                            # store RAW denominator rows to DRAM, broadcast
                            # them across 64 partitions (zero-stride APs need a
                            # DRAM source), then take the fast reciprocal on the
                            # broadcast tile (64 lanes) - shortest serial chain
                            for h in range(2):
                                nc.sync.dma_start(
                                    den_dram[2 * hp + h][None,
                                                         qc * 512:(qc + 1) * 512],
                                    asbs[h][HD:HD + 1, :],
                                )
                            for h in range(2):
                                rb = norm.tile([HD, 512], f32, tag=f"rb{h}",
                                               name=f"rb{h}")
                                bcast_ap = bass.AP(
                                    tensor=den_dram,
                                    offset=(hp * 2 + h) * N + qc * 512,
                                    ap=[[0, HD], [1, 512]],
                                )
                                nc.gpsimd.dma_start(out=rb[:], in_=bcast_ap)
                                nc.vector.reciprocal_approx_fast(rb[:], rb[:])
                                nc.vector.tensor_mul(
                                    outT[h * 64:(h + 1) * 64, hp,
                                         qc * 512:(qc + 1) * 512],
                                    asbs[h][0:HD, :],
                                    rb[:],
                                )
                        # drain any remaining qk steps for the next head pair
                        for step in pending:
                            step()

                    # ---- remaining output projection (token tiles 4..7),
                    #      still inside the attention pools: double-buffer
                    #      through the freed s_ps slots (fc halves land in
                    #      separate banks of a [128,1024] tile) ----
                    for tt in range(4, NT):
                        # alternate psum pools for 4-deep tt pipelining
                        if tt % 2 == 0:
                            psy = s_ps.tile([128, 1024], f32, tag="s",
                                            name="psy")
                            halves = (psy[:, 0:384], psy[:, 512:896])
                        else:
                            psya = qk_ps.tile([128, 512], f32, tag="psq0",
                                              name="psya")
                            psyb = qk_ps.tile([128, 512], f32, tag="psq1",
                                              name="psyb")
                            halves = (psya[:, 0:384], psyb[:, 0:384])
                        for j in range(NHP):
                            for fc in range(2):
                                nc.tensor.matmul(
                                    halves[fc],
                                    outT[:, j, tt * 128:(tt + 1) * 128],
                                    wp_sb[:, j, fc * 384:(fc + 1) * 384],
                                    start=(j == 0),
                                    stop=(j == NHP - 1),
                                )
                        for fc in range(2):
                            # fused evict + bias add
                            yst = ystage.tile([128, 384], f32, tag="yst",
                                              name="yst")
                            nc.vector.tensor_add(
                                yst[:], halves[fc],
                                bias_bc[:, fc * 384:(fc + 1) * 384],
                            )
                            nc.sync.dma_start(
                                y[tt * 128:(tt + 1) * 128,
                                  fc * 384:(fc + 1) * 384],
                                yst[:],
                            )

    nc.compile()
    return nc


def kernel(**inputs) -> np.ndarray:
    from concourse.bass_utils import run_bass_kernel_spmd

    x = np.ascontiguousarray(np.asarray(inputs["x"], dtype=np.float32))
    wqkv = np.ascontiguousarray(np.asarray(inputs["W_qkv"], dtype=np.float32))
    wproj = np.ascontiguousarray(np.asarray(inputs["W_proj"], dtype=np.float32))
    bproj = np.ascontiguousarray(np.asarray(inputs["b_proj"], dtype=np.float32))

    if "nc" not in _STATE:
        _STATE["nc"] = _build()
    nc = _STATE["nc"]

    in_maps = [
        {"x": x[b], "w_qkv": wqkv, "w_proj": wproj, "b_proj": bproj}
        for b in range(B)
    ]
    res = run_bass_kernel_spmd(nc, in_maps, list(range(B)))
    out = np.stack([res.results[b]["y"] for b in range(B)], axis=0)
    return out.astype(np.float32)
